# revision 23
# baseline (speedup 1.0000x reference)
"""
nn_BiReBlock kernel for 8x Trainium2 NeuronCores.

Mathematical reduction (same as the verified baseline)
------------------------------------------------------
reference(X, W) with W having orthonormal rows reduces to
    out = Wm @ X @ Wm^T + eps * diag(1_N)
where Wm = W with QR-sign-negative rows zeroed (for the actual seed-0 W,
QR reproduces W exactly so Wm = W, N = {}).

Device computation (v8, "packed half-staircase")
------------------------------------------------
* fp16 is plenty (2e-2 budget vs ~4e-4 measured end-to-end), no residual.
* X is symmetric, so only the lower 2x2 block staircase of it is shipped:
  L' = [[X11/2, 0], [X21, X22/2]], which satisfies L' + L'^T = X.
  Since S = Wm X Wm^T is symmetric the device computes
      Z_b = Wm L'_b^T Wm^T
  and the host reconstructs S = Z + Z^T for free (75% of X shipped).
* The left block-column [128, 64] is a dense per-item stationary.
  The right-bottom blocks [64, 64] of consecutive item pairs are packed
  onto all 128 SBUF partitions (even item on 0:64, odd on 64:128), so
  every DMA runs with full 128-partition parallelism; the other item is
  masked out by a zero half in the W^T moving operand (sections 2/3 of
  WH3), so no SBUF zero-fills or row-tiled matmuls are needed and the
  PE's weight-load pull-ahead keeps streaming at full rate.
* stage 2 packs two 8-item groups into PSUM partitions 0:64 / 64:128
  via column tiling; it is emitted two half-groups late (software
  pipelining) so its wait for the PSUM->SBUF copy never stalls the
  in-order PE.  Copies convert to fp16 and alternate between the
  Vector and Scalar engines; output is fp16.
* A burst of dummy matmuls right after the first DMAs keeps the PE HAM
  activity window busy, so real work starts at the warm 2.4 GHz clock.

HBM traffic/core: 12 MB X + 4 MB out (vs 32 MB baseline).
"""

import numpy as np

B_TOTAL = 4096
N_CORES = 8
B_LOCAL = B_TOTAL // N_CORES
D_IN = 128
D_OUT = 64
EPS = 1e-4

_CACHE = {}

CHUNKS = [16, 16] + [32] * 15
assert sum(CHUNKS) == B_LOCAL
XCH_MAX = max(CHUNKS)
NXBUF = 6
GROUP = 8
PAIR = 2 * GROUP          # items per packed stage-2 PSUM bank
OCH = 32                  # items per output flush
H = D_IN // 2             # 64
DEFER = 2                 # half-groups of stage-2 deferral


def _build_nc(b_local):
    import concourse.tile as tile
    from concourse import bacc, mybir

    f32 = mybir.dt.float32
    f16 = mybir.dt.float16
    nc = bacc.Bacc(None, target_bir_lowering=False)

    # left block-column of L', i-major: XL[i, b*64 + j] = L'_b[i, j]
    xld = nc.dram_tensor("XL", [D_IN, b_local * H], f16, kind="ExternalInput")
    # right-bottom blocks, item-pair packed:
    # XR[s*64 + p, q*64 + j] = (X22/2)_{2q+s}[p, j]
    xrd = nc.dram_tensor("XR", [D_IN, (b_local // 2) * H], f16,
                         kind="ExternalInput")
    # [W^T | W^T | [W^T[64:]; 0] | [0; W^T[64:]]] — the zero halves of the
    # last two sections mask out the other item of each packed XR pair
    wd = nc.dram_tensor("WH3", [D_IN, 4 * D_OUT], f16, kind="ExternalInput")
    n_pair = b_local // PAIR
    outd = nc.dram_tensor("OUT", [D_IN, n_pair * GROUP * D_OUT], f16,
                          kind="ExternalOutput")

    gfree = GROUP * D_OUT   # 512

    with tile.TileContext(nc) as tc:
        with (
            tc.tile_pool(name="const", bufs=1) as cpool,
            tc.tile_pool(name="tsb", bufs=5) as tpool,
            tc.tile_pool(name="obuf", bufs=3) as opool,
            tc.tile_pool(name="psum_t", bufs=5, space="PSUM") as pt,
            tc.tile_pool(name="psum_s", bufs=3, space="PSUM") as ps,
        ):
            wh3 = cpool.tile([D_IN, 4 * D_OUT], f16)
            nc.sync.dma_start(wh3[:], wd[:])

            # fixed X tiles (NXBUF buffers x {left, right}), rotated manually
            xbufs = [
                (cpool.tile([D_IN, XCH_MAX, H], f16, name=f"xl{i}"),
                 cpool.tile([D_IN, XCH_MAX // 2, H], f16, name=f"xr{i}"))
                for i in range(NXBUF)
            ]

            chunk_base = np.cumsum([0] + CHUNKS).tolist()

            def issue_chunk(k):
                b0, n = chunk_base[k], CHUNKS[k]
                xl, xr = xbufs[k % NXBUF]
                nc.sync.dma_start(
                    xl[:, 0:n, :], xld[:, b0 * H : (b0 + n) * H]
                )
                nc.gpsimd.dma_start(
                    xr[:, 0 : n // 2, :],
                    xrd[:, (b0 // 2) * H : ((b0 + n) // 2) * H],
                )

            # chunk k+NXBUF-1 is issued only once chunk k's compute emission
            # begins, so WAR edges against the buffer's previous user are
            # ordered correctly.
            for k in range(min(NXBUF - 1, len(CHUNKS))):
                issue_chunk(k)

            # HAM warm-up: keep the PE busy on dummy matmuls while the
            # first chunks are still in flight (only wh3 is needed).
            warm = ps.tile([D_IN, gfree], f32, tag="sp", name="warm")
            for r in range(112):
                half = (r // 8) % 2
                nc.tensor.matmul(
                    warm[half * D_OUT : (half + 1) * D_OUT,
                         (r % 8) * D_OUT : (r % 8 + 1) * D_OUT],
                    wh3[:, 0:D_OUT],
                    wh3[:, 0:D_OUT],
                    start=True,
                    stop=True,
                    tile_position=(0, half * D_OUT),
                )

            obufs = {}
            sps = {}

            def emit_stage2(pg, h, ts):
                if h == 0:
                    sps[pg] = ps.tile([D_IN, gfree], f32, tag="sp", name="sp")
                sp = sps[pg]
                nc.tensor.matmul(
                    sp[h * D_OUT : (h + 1) * D_OUT, :],
                    wh3[:, h * D_OUT : (h + 1) * D_OUT],
                    ts[:],
                    start=True,
                    stop=True,
                    tile_position=(0, h * D_OUT),
                )
                if h == 0:
                    return
                del sps[pg]
                obuf = obufs[pg // (OCH // PAIR)]
                off = (pg % (OCH // PAIR)) * gfree
                scpy = nc.scalar.copy if pg % 2 == 0 else nc.vector.tensor_copy
                scpy(obuf[:, off : off + gfree], sp[:])
                c0 = pg * PAIR
                if (c0 + PAIR) % OCH == 0:
                    o0 = (pg // (OCH // PAIR)) * (OCH // PAIR) * gfree
                    olen = (OCH // PAIR) * gfree
                    if c0 + PAIR == b_local:
                        nc.scalar.dma_start(
                            outd[:, o0 : o0 + olen // 2], obuf[:, : olen // 2]
                        )
                        nc.scalar.dma_start(
                            outd[:, o0 + olen // 2 : o0 + olen],
                            obuf[:, olen // 2 :],
                        )
                    else:
                        nc.scalar.dma_start(outd[:, o0 : o0 + olen], obuf[:])

            queue = []  # (pg, h, ts) awaiting stage-2
            for pg in range(b_local // PAIR):
                c0 = pg * PAIR
                k = next(i for i in range(len(CHUNKS))
                         if chunk_base[i] <= c0 < chunk_base[i + 1])
                xl, xr = xbufs[k % NXBUF]
                if c0 == chunk_base[k] and k + NXBUF - 1 < len(CHUNKS):
                    issue_chunk(k + NXBUF - 1)
                if c0 % OCH == 0:
                    obufs[pg // (OCH // PAIR)] = opool.tile(
                        [D_IN, (OCH // PAIR) * gfree], f16, tag="obuf",
                        name="obuf",
                    )
                for h in range(2):
                    tp = pt.tile([D_IN, gfree], f32, name="tp")
                    for j in range(GROUP):
                        b = c0 - chunk_base[k] + h * GROUP + j
                        dst = tp[:, j * D_OUT : (j + 1) * D_OUT]
                        nc.tensor.matmul(
                            dst[0:H, :],
                            xl[:, b, :],
                            wh3[:, 0:D_OUT],
                            start=True,
                            stop=True,
                            tile_position=(0, 0),
                        )
                        s = b % 2
                        nc.tensor.matmul(
                            dst[H:D_IN, :],
                            xr[:, b // 2, :],
                            wh3[:, (2 + s) * D_OUT : (3 + s) * D_OUT],
                            start=True,
                            stop=True,
                            tile_position=(0, H),
                        )
                    ts = tpool.tile([D_IN, gfree], f16, tag="ts", name="ts")
                    cpy = (nc.vector.tensor_copy if (2 * pg + h) % 2 == 0
                           else nc.scalar.copy)
                    cpy(ts[:], tp[:])
                    queue.append((pg, h, ts))
                    if len(queue) > DEFER:
                        emit_stage2(*queue.pop(0))
            while queue:
                emit_stage2(*queue.pop(0))

    nc.compile()
    return nc


def _get_nc(b_local):
    if b_local not in _CACHE:
        _CACHE[b_local] = _build_nc(b_local)
    return _CACHE[b_local]


def _host_prep(W):
    """Derive the sign diagonal of the reference's QR and the masked W.

    Returns (wm, d) or (None, None) when W doesn't have orthonormal rows
    (then the closed form doesn't apply and the caller falls back)."""
    W = np.ascontiguousarray(W, dtype=np.float32)
    q, _ = np.linalg.qr(W.T)
    d = np.sign((q.T * W).sum(axis=1)).astype(np.float32)
    d[d == 0] = 1.0
    if np.abs(q.T - d[:, None] * W).max() >= 1e-4:
        return None, None
    wm = W * (d > 0).astype(np.float32)[:, None]
    return wm, d


def _reference_fallback(X, W):
    """Faithful numpy port of the reference (QR + eigh) — only used if the
    input W unexpectedly doesn't have orthonormal rows."""
    q, _ = np.linalg.qr(W.T.astype(np.float32))
    w_st = q.T
    y = np.einsum("mi,bij->bmj", w_st, X, optimize=True) @ W.T
    m = 0.5 * (y + y.transpose(0, 2, 1))
    lam, u = np.linalg.eigh(m)
    lam = np.maximum(lam, EPS)
    return np.einsum("bik,bk,bjk->bij", u, lam, u, optimize=True).astype(np.float32)


def run(X, W, trace=False, **trace_kwargs):
    X = np.ascontiguousarray(X, dtype=np.float32)
    wm, d = _host_prep(W)
    if wm is None:
        return _reference_fallback(X, W), None

    wh = wm.T.astype(np.float16)  # [128, 64] = W^T
    zz = np.zeros((H, D_OUT), dtype=np.float16)
    whe = np.concatenate([wh[H:D_IN], zz], axis=0)  # masks odd item
    who = np.concatenate([zz, wh[H:D_IN]], axis=0)  # masks even item
    wh3 = np.concatenate([wh, wh, whe, who], axis=1)  # [128, 256]

    # [B, i, b, j] i-major fp16
    xh = X.astype(np.float16)
    xh = xh.reshape(N_CORES, B_LOCAL, D_IN, D_IN).transpose(0, 2, 1, 3)
    # left block-column of L' (top 64x64 block halved; exact in fp16)
    xl = np.ascontiguousarray(xh[:, :, :, 0:H])
    xl[:, 0:H, :, :] *= np.float16(0.5)
    xl = xl.reshape(N_CORES, D_IN, B_LOCAL * H)
    # right-bottom block X22/2, item-pair packed onto 128 partitions
    xr = xh[:, H:D_IN, :, H:D_IN] * np.float16(0.5)  # [core, 64, b, 64]
    xr = xr.reshape(N_CORES, H, B_LOCAL // 2, 2, H).transpose(0, 3, 1, 2, 4)
    xr = np.ascontiguousarray(xr).reshape(N_CORES, D_IN, (B_LOCAL // 2) * H)

    from concourse.bass_utils import run_bass_kernel_spmd

    nc = _get_nc(B_LOCAL)
    in_maps = [
        {"XL": xl[c], "XR": xr[c], "WH3": wh3} for c in range(N_CORES)
    ]
    last_err = None
    for _attempt in range(3):
        try:
            res = run_bass_kernel_spmd(
                nc, in_maps, list(range(N_CORES)), trace=trace, **trace_kwargs
            )
            break
        except Exception as e:  # noqa: BLE001 - transient NRT device errors
            last_err = e
            import time

            time.sleep(2.0)
    else:
        raise last_err

    n_pair = B_LOCAL // PAIR
    z = np.empty((B_TOTAL, D_OUT, D_OUT), dtype=np.float32)
    for c in range(N_CORES):
        o = res.results[c]["OUT"].reshape(2, D_OUT, n_pair, GROUP, D_OUT)
        # o[h, m, pg, j, n] = Z[pg*16 + h*8 + j][m, n]
        o = o.transpose(2, 0, 3, 1, 4).reshape(B_LOCAL, D_OUT, D_OUT)
        z[c * B_LOCAL : (c + 1) * B_LOCAL] = o
    out = z + z.transpose(0, 2, 1)  # S = Z + Z^T (L' + L'^T = X)
    neg = d < 0
    if neg.any():
        idx = np.where(neg)[0]
        out[:, idx, idx] += EPS
    return out, res


def kernel(X, W):
    return run(X, W)[0]


# revision 28
# speedup vs baseline: 1.1553x; 1.1553x over previous
"""
nn_BiReBlock kernel for 8x Trainium2 NeuronCores.

Mathematical reduction (same as the verified baseline)
------------------------------------------------------
reference(X, W) with W having orthonormal rows reduces to
    out = Wm @ X @ Wm^T + eps * diag(1_N)
where Wm = W with QR-sign-negative rows zeroed (for the actual seed-0 W,
QR reproduces W exactly so Wm = W, N = {}).

Device computation (v8, "packed half-staircase")
------------------------------------------------
* fp16 is plenty (2e-2 budget vs ~4e-4 measured end-to-end), no residual.
* X is symmetric, so only the lower 2x2 block staircase of it is shipped:
  L' = [[X11/2, 0], [X21, X22/2]], which satisfies L' + L'^T = X.
  Since S = Wm X Wm^T is symmetric the device computes
      Z_b = Wm L'_b^T Wm^T
  and the host reconstructs S = Z + Z^T for free (75% of X shipped).
* The left block-column [128, 64] is a dense per-item stationary.
  The right-bottom blocks [64, 64] of consecutive item pairs are packed
  onto all 128 SBUF partitions (even item on 0:64, odd on 64:128), so
  every DMA runs with full 128-partition parallelism; the other item is
  masked out by a zero half in the W^T moving operand (sections 2/3 of
  WH3), so no SBUF zero-fills or row-tiled matmuls are needed and the
  PE's weight-load pull-ahead keeps streaming at full rate.
* stage 2 packs two 8-item groups into PSUM partitions 0:64 / 64:128
  via column tiling; it is emitted two half-groups late (software
  pipelining) so its wait for the PSUM->SBUF copy never stalls the
  in-order PE.  Copies convert to fp16 and alternate between the
  Vector and Scalar engines; output is fp16.
* A burst of dummy matmuls right after the first DMAs keeps the PE HAM
  activity window busy, so real work starts at the warm 2.4 GHz clock.

HBM traffic/core: 12 MB X + 4 MB out (vs 32 MB baseline).
"""

import numpy as np

B_TOTAL = 4096
N_CORES = 8
B_LOCAL = B_TOTAL // N_CORES
D_IN = 128
D_OUT = 64
EPS = 1e-4

_CACHE = {}

CHUNKS = [16, 16, 32, 64] + [96] * 4
assert sum(CHUNKS) == B_LOCAL
GROUP = 8
PAIR = 2 * GROUP          # items per packed stage-2 PSUM bank
OCH = 32                  # items per output flush
H = D_IN // 2             # 64
DEFER = 2                 # half-groups of stage-2 deferral


def _build_nc(b_local):
    import concourse.tile as tile
    from concourse import bacc, mybir

    f32 = mybir.dt.float32
    f16 = mybir.dt.float16
    nc = bacc.Bacc(None, target_bir_lowering=False)

    # left block-column of L', i-major: XL[i, b*64 + j] = L'_b[i, j]
    xld = nc.dram_tensor("XL", [D_IN, b_local * H], f16, kind="ExternalInput")
    # right-bottom blocks, item-pair packed:
    # XR[s*64 + p, q*64 + j] = (X22/2)_{2q+s}[p, j]
    xrd = nc.dram_tensor("XR", [D_IN, (b_local // 2) * H], f16,
                         kind="ExternalInput")
    # [W^T | W^T | [W^T[64:]; 0] | [0; W^T[64:]]] — the zero halves of the
    # last two sections mask out the other item of each packed XR pair
    wd = nc.dram_tensor("WH3", [D_IN, 4 * D_OUT], f16, kind="ExternalInput")
    n_pair = b_local // PAIR
    outd = nc.dram_tensor("OUT", [D_IN, n_pair * GROUP * D_OUT], f16,
                          kind="ExternalOutput")

    gfree = GROUP * D_OUT   # 512

    with tile.TileContext(nc) as tc:
        with (
            tc.tile_pool(name="const", bufs=1) as cpool,
            tc.tile_pool(name="tsb", bufs=5) as tpool,
            tc.tile_pool(name="obuf", bufs=3) as opool,
            tc.tile_pool(name="psum_t", bufs=5, space="PSUM") as pt,
            tc.tile_pool(name="psum_s", bufs=3, space="PSUM") as ps,
        ):
            wh3 = cpool.tile([D_IN, 4 * D_OUT], f16)
            nc.sync.dma_start(wh3[:], wd[:])

            # the whole per-core input fits in SBUF (96 KB/partition), so
            # keep it all resident and pre-issue every input DMA up front
            # in consumption order — no rotation, no WAR-gated prefetch
            xl = cpool.tile([D_IN, b_local, H], f16, name="xl")
            xr = cpool.tile([D_IN, b_local // 2, H], f16, name="xr")

            chunk_base = np.cumsum([0] + CHUNKS).tolist()
            for k in range(len(CHUNKS)):
                b0, n = chunk_base[k], CHUNKS[k]
                nc.sync.dma_start(
                    xl[:, b0 : b0 + n, :], xld[:, b0 * H : (b0 + n) * H]
                )
                nc.sync.dma_start(
                    xr[:, b0 // 2 : (b0 + n) // 2, :],
                    xrd[:, (b0 // 2) * H : ((b0 + n) // 2) * H],
                )

            # HAM warm-up: keep the PE busy on dummy matmuls while the
            # first chunks are still in flight (only wh3 is needed).
            warm = ps.tile([D_IN, gfree], f32, tag="sp", name="warm")
            for r in range(112):
                half = (r // 8) % 2
                nc.tensor.matmul(
                    warm[half * D_OUT : (half + 1) * D_OUT,
                         (r % 8) * D_OUT : (r % 8 + 1) * D_OUT],
                    wh3[:, 0:D_OUT],
                    wh3[:, 0:D_OUT],
                    start=True,
                    stop=True,
                    tile_position=(0, half * D_OUT),
                )

            obufs = {}
            sps = {}

            def emit_stage2(pg, h, ts):
                if h == 0:
                    sps[pg] = ps.tile([D_IN, gfree], f32, tag="sp", name="sp")
                sp = sps[pg]
                nc.tensor.matmul(
                    sp[h * D_OUT : (h + 1) * D_OUT, :],
                    wh3[:, h * D_OUT : (h + 1) * D_OUT],
                    ts[:],
                    start=True,
                    stop=True,
                    tile_position=(0, h * D_OUT),
                )
                if h == 0:
                    return
                del sps[pg]
                obuf = obufs[pg // (OCH // PAIR)]
                off = (pg % (OCH // PAIR)) * gfree
                scpy = nc.scalar.copy if pg % 2 == 0 else nc.vector.tensor_copy
                scpy(obuf[:, off : off + gfree], sp[:])
                c0 = pg * PAIR
                if (c0 + PAIR) % OCH == 0:
                    o0 = (pg // (OCH // PAIR)) * (OCH // PAIR) * gfree
                    olen = (OCH // PAIR) * gfree
                    if c0 + PAIR == b_local:
                        nc.gpsimd.dma_start(
                            outd[:, o0 : o0 + olen // 2], obuf[:, : olen // 2]
                        )
                        nc.gpsimd.dma_start(
                            outd[:, o0 + olen // 2 : o0 + olen],
                            obuf[:, olen // 2 :],
                        )
                    else:
                        nc.gpsimd.dma_start(outd[:, o0 : o0 + olen], obuf[:])

            queue = []  # (pg, h, ts) awaiting stage-2
            for pg in range(b_local // PAIR):
                c0 = pg * PAIR
                if c0 % OCH == 0:
                    obufs[pg // (OCH // PAIR)] = opool.tile(
                        [D_IN, (OCH // PAIR) * gfree], f16, tag="obuf",
                        name="obuf",
                    )
                for h in range(2):
                    tp = pt.tile([D_IN, gfree], f32, name="tp")
                    for j in range(GROUP):
                        b = c0 + h * GROUP + j
                        dst = tp[:, j * D_OUT : (j + 1) * D_OUT]
                        nc.tensor.matmul(
                            dst[0:H, :],
                            xl[:, b, :],
                            wh3[:, 0:D_OUT],
                            start=True,
                            stop=True,
                            tile_position=(0, 0),
                        )
                        s = b % 2
                        nc.tensor.matmul(
                            dst[H:D_IN, :],
                            xr[:, b // 2, :],
                            wh3[:, (2 + s) * D_OUT : (3 + s) * D_OUT],
                            start=True,
                            stop=True,
                            tile_position=(0, H),
                        )
                    ts = tpool.tile([D_IN, gfree], f16, tag="ts", name="ts")
                    cpy = (nc.vector.tensor_copy if (2 * pg + h) % 2 == 0
                           else nc.scalar.copy)
                    cpy(ts[:], tp[:])
                    queue.append((pg, h, ts))
                    if len(queue) > DEFER:
                        emit_stage2(*queue.pop(0))
            while queue:
                emit_stage2(*queue.pop(0))

    nc.compile()
    return nc


def _get_nc(b_local):
    if b_local not in _CACHE:
        _CACHE[b_local] = _build_nc(b_local)
    return _CACHE[b_local]


def _host_prep(W):
    """Derive the sign diagonal of the reference's QR and the masked W.

    Returns (wm, d) or (None, None) when W doesn't have orthonormal rows
    (then the closed form doesn't apply and the caller falls back)."""
    W = np.ascontiguousarray(W, dtype=np.float32)
    q, _ = np.linalg.qr(W.T)
    d = np.sign((q.T * W).sum(axis=1)).astype(np.float32)
    d[d == 0] = 1.0
    if np.abs(q.T - d[:, None] * W).max() >= 1e-4:
        return None, None
    wm = W * (d > 0).astype(np.float32)[:, None]
    return wm, d


def _reference_fallback(X, W):
    """Faithful numpy port of the reference (QR + eigh) — only used if the
    input W unexpectedly doesn't have orthonormal rows."""
    q, _ = np.linalg.qr(W.T.astype(np.float32))
    w_st = q.T
    y = np.einsum("mi,bij->bmj", w_st, X, optimize=True) @ W.T
    m = 0.5 * (y + y.transpose(0, 2, 1))
    lam, u = np.linalg.eigh(m)
    lam = np.maximum(lam, EPS)
    return np.einsum("bik,bk,bjk->bij", u, lam, u, optimize=True).astype(np.float32)


def run(X, W, trace=False, **trace_kwargs):
    X = np.ascontiguousarray(X, dtype=np.float32)
    wm, d = _host_prep(W)
    if wm is None:
        return _reference_fallback(X, W), None

    wh = wm.T.astype(np.float16)  # [128, 64] = W^T
    zz = np.zeros((H, D_OUT), dtype=np.float16)
    whe = np.concatenate([wh[H:D_IN], zz], axis=0)  # masks odd item
    who = np.concatenate([zz, wh[H:D_IN]], axis=0)  # masks even item
    wh3 = np.concatenate([wh, wh, whe, who], axis=1)  # [128, 256]

    # [B, i, b, j] i-major fp16
    xh = X.astype(np.float16)
    xh = xh.reshape(N_CORES, B_LOCAL, D_IN, D_IN).transpose(0, 2, 1, 3)
    # left block-column of L' (top 64x64 block halved; exact in fp16)
    xl = np.ascontiguousarray(xh[:, :, :, 0:H])
    xl[:, 0:H, :, :] *= np.float16(0.5)
    xl = xl.reshape(N_CORES, D_IN, B_LOCAL * H)
    # right-bottom block X22/2, item-pair packed onto 128 partitions
    xr = xh[:, H:D_IN, :, H:D_IN] * np.float16(0.5)  # [core, 64, b, 64]
    xr = xr.reshape(N_CORES, H, B_LOCAL // 2, 2, H).transpose(0, 3, 1, 2, 4)
    xr = np.ascontiguousarray(xr).reshape(N_CORES, D_IN, (B_LOCAL // 2) * H)

    from concourse.bass_utils import run_bass_kernel_spmd

    nc = _get_nc(B_LOCAL)
    in_maps = [
        {"XL": xl[c], "XR": xr[c], "WH3": wh3} for c in range(N_CORES)
    ]
    last_err = None
    for _attempt in range(3):
        try:
            res = run_bass_kernel_spmd(
                nc, in_maps, list(range(N_CORES)), trace=trace, **trace_kwargs
            )
            break
        except Exception as e:  # noqa: BLE001 - transient NRT device errors
            last_err = e
            import time

            time.sleep(2.0)
    else:
        raise last_err

    n_pair = B_LOCAL // PAIR
    z = np.empty((B_TOTAL, D_OUT, D_OUT), dtype=np.float32)
    for c in range(N_CORES):
        o = res.results[c]["OUT"].reshape(2, D_OUT, n_pair, GROUP, D_OUT)
        # o[h, m, pg, j, n] = Z[pg*16 + h*8 + j][m, n]
        o = o.transpose(2, 0, 3, 1, 4).reshape(B_LOCAL, D_OUT, D_OUT)
        z[c * B_LOCAL : (c + 1) * B_LOCAL] = o
    out = z + z.transpose(0, 2, 1)  # S = Z + Z^T (L' + L'^T = X)
    neg = d < 0
    if neg.any():
        idx = np.where(neg)[0]
        out[:, idx, idx] += EPS
    return out, res


def kernel(X, W):
    return run(X, W)[0]


# revision 30
# speedup vs baseline: 1.1629x; 1.0066x over previous
"""
nn_BiReBlock kernel for 8x Trainium2 NeuronCores.

Mathematical reduction (same as the verified baseline)
------------------------------------------------------
reference(X, W) with W having orthonormal rows reduces to
    out = Wm @ X @ Wm^T + eps * diag(1_N)
where Wm = W with QR-sign-negative rows zeroed (for the actual seed-0 W,
QR reproduces W exactly so Wm = W, N = {}).

Device computation (v8, "packed half-staircase")
------------------------------------------------
* fp16 is plenty (2e-2 budget vs ~4e-4 measured end-to-end), no residual.
* X is symmetric, so only the lower 2x2 block staircase of it is shipped:
  L' = [[X11/2, 0], [X21, X22/2]], which satisfies L' + L'^T = X.
  Since S = Wm X Wm^T is symmetric the device computes
      Z_b = Wm L'_b^T Wm^T
  and the host reconstructs S = Z + Z^T for free (75% of X shipped).
* The left block-column [128, 64] is a dense per-item stationary.
  The right-bottom blocks [64, 64] of consecutive item pairs are packed
  onto all 128 SBUF partitions (even item on 0:64, odd on 64:128), so
  every DMA runs with full 128-partition parallelism; the other item is
  masked out by a zero half in the W^T moving operand (sections 2/3 of
  WH3), so no SBUF zero-fills or row-tiled matmuls are needed and the
  PE's weight-load pull-ahead keeps streaming at full rate.
* stage 2 packs two 8-item groups into PSUM partitions 0:64 / 64:128
  via column tiling; it is emitted two half-groups late (software
  pipelining) so its wait for the PSUM->SBUF copy never stalls the
  in-order PE.  Copies convert to fp16 and alternate between the
  Vector and Scalar engines; output is fp16.
* A burst of dummy matmuls right after the first DMAs keeps the PE HAM
  activity window busy, so real work starts at the warm 2.4 GHz clock.

HBM traffic/core: 12 MB X + 4 MB out (vs 32 MB baseline).
"""

import numpy as np

B_TOTAL = 4096
N_CORES = 8
B_LOCAL = B_TOTAL // N_CORES
D_IN = 128
D_OUT = 64
EPS = 1e-4

_CACHE = {}

CHUNKS = [16, 16, 32, 64] + [96] * 4
assert sum(CHUNKS) == B_LOCAL
GROUP = 8
PAIR = 2 * GROUP          # items per packed stage-2 PSUM bank
OCH = 32                  # items per output flush
H = D_IN // 2             # 64
DEFER = 2                 # half-groups of stage-2 deferral


def _build_nc(b_local):
    import concourse.tile as tile
    from concourse import bacc, mybir

    f32 = mybir.dt.float32
    f16 = mybir.dt.float16
    nc = bacc.Bacc(None, target_bir_lowering=False)

    # left block-column of L', i-major: XL[i, b*64 + j] = L'_b[i, j]
    xld = nc.dram_tensor("XL", [D_IN, b_local * H], f16, kind="ExternalInput")
    # right-bottom blocks, item-pair packed:
    # XR[s*64 + p, q*64 + j] = (X22/2)_{2q+s}[p, j]
    xrd = nc.dram_tensor("XR", [D_IN, (b_local // 2) * H], f16,
                         kind="ExternalInput")
    # [W^T | W^T | [W^T[64:]; 0] | [0; W^T[64:]]] — the zero halves of the
    # last two sections mask out the other item of each packed XR pair
    wd = nc.dram_tensor("WH3", [D_IN, 4 * D_OUT], f16, kind="ExternalInput")
    n_pair = b_local // PAIR
    outd = nc.dram_tensor("OUT", [D_IN, n_pair * GROUP * D_OUT], f16,
                          kind="ExternalOutput")

    gfree = GROUP * D_OUT   # 512

    with tile.TileContext(nc) as tc:
        with (
            tc.tile_pool(name="const", bufs=1) as cpool,
            tc.tile_pool(name="tsb", bufs=5) as tpool,
            tc.tile_pool(name="obuf", bufs=3) as opool,
            tc.tile_pool(name="psum_t", bufs=4, space="PSUM") as pt,
            tc.tile_pool(name="psum_s", bufs=4, space="PSUM") as ps,
        ):
            wh3 = cpool.tile([D_IN, 4 * D_OUT], f16)
            nc.sync.dma_start(wh3[:], wd[:])

            # the whole per-core input fits in SBUF (96 KB/partition), so
            # keep it all resident and pre-issue every input DMA up front
            # in consumption order — no rotation, no WAR-gated prefetch
            xl = cpool.tile([D_IN, b_local, H], f16, name="xl")
            xr = cpool.tile([D_IN, b_local // 2, H], f16, name="xr")

            chunk_base = np.cumsum([0] + CHUNKS).tolist()
            for k in range(len(CHUNKS)):
                b0, n = chunk_base[k], CHUNKS[k]
                nc.sync.dma_start(
                    xl[:, b0 : b0 + n, :], xld[:, b0 * H : (b0 + n) * H]
                )
                nc.sync.dma_start(
                    xr[:, b0 // 2 : (b0 + n) // 2, :],
                    xrd[:, (b0 // 2) * H : ((b0 + n) // 2) * H],
                )

            # HAM warm-up: keep the PE busy on dummy matmuls while the
            # first chunks are still in flight (only wh3 is needed).
            warm = ps.tile([D_IN, gfree], f32, tag="sp", name="warm")
            for r in range(112):
                half = (r // 8) % 2
                nc.tensor.matmul(
                    warm[half * D_OUT : (half + 1) * D_OUT,
                         (r % 8) * D_OUT : (r % 8 + 1) * D_OUT],
                    wh3[:, 0:D_OUT],
                    wh3[:, 0:D_OUT],
                    start=True,
                    stop=True,
                    tile_position=(0, half * D_OUT),
                )

            obufs = {}
            sps = {}

            def emit_stage2(pg, h, ts):
                if h == 0:
                    sps[pg] = ps.tile([D_IN, gfree], f32, tag="sp", name="sp")
                sp = sps[pg]
                nc.tensor.matmul(
                    sp[h * D_OUT : (h + 1) * D_OUT, :],
                    wh3[:, h * D_OUT : (h + 1) * D_OUT],
                    ts[:],
                    start=True,
                    stop=True,
                    tile_position=(0, h * D_OUT),
                )

            def emit_scopy(pg):
                sp = sps.pop(pg)
                obuf = obufs[pg // (OCH // PAIR)]
                off = (pg % (OCH // PAIR)) * gfree
                scpy = nc.scalar.copy if pg % 2 == 0 else nc.vector.tensor_copy
                scpy(obuf[:, off : off + gfree], sp[:])
                c0 = pg * PAIR
                if (c0 + PAIR) % OCH == 0:
                    o0 = (pg // (OCH // PAIR)) * (OCH // PAIR) * gfree
                    olen = (OCH // PAIR) * gfree
                    if c0 + PAIR == b_local:
                        nc.gpsimd.dma_start(
                            outd[:, o0 : o0 + olen // 2], obuf[:, : olen // 2]
                        )
                        nc.gpsimd.dma_start(
                            outd[:, o0 + olen // 2 : o0 + olen],
                            obuf[:, olen // 2 :],
                        )
                    else:
                        nc.gpsimd.dma_start(outd[:, o0 : o0 + olen], obuf[:])

            queue = []    # (pg, h, ts) awaiting stage-2
            s_queue = []  # pg awaiting the sp -> obuf copy
            for pg in range(b_local // PAIR):
                c0 = pg * PAIR
                if c0 % OCH == 0:
                    obufs[pg // (OCH // PAIR)] = opool.tile(
                        [D_IN, (OCH // PAIR) * gfree], f16, tag="obuf",
                        name="obuf",
                    )
                for h in range(2):
                    tp = pt.tile([D_IN, gfree], f32, name="tp")
                    for j in range(GROUP):
                        b = c0 + h * GROUP + j
                        dst = tp[:, j * D_OUT : (j + 1) * D_OUT]
                        nc.tensor.matmul(
                            dst[0:H, :],
                            xl[:, b, :],
                            wh3[:, 0:D_OUT],
                            start=True,
                            stop=True,
                            tile_position=(0, 0),
                        )
                        s = b % 2
                        nc.tensor.matmul(
                            dst[H:D_IN, :],
                            xr[:, b // 2, :],
                            wh3[:, (2 + s) * D_OUT : (3 + s) * D_OUT],
                            start=True,
                            stop=True,
                            tile_position=(0, H),
                        )
                    ts = tpool.tile([D_IN, gfree], f16, tag="ts", name="ts")
                    cpy = (nc.vector.tensor_copy if (2 * pg + h) % 2 == 0
                           else nc.scalar.copy)
                    cpy(ts[:], tp[:])
                    queue.append((pg, h, ts))
                    if len(queue) > DEFER:
                        qpg, qh, qts = queue.pop(0)
                        emit_stage2(qpg, qh, qts)
                        if qh == 1:
                            s_queue.append(qpg)
                    # S-copies trail stage-2 by two more halves so a
                    # not-yet-ready S-copy never head-of-line-blocks the
                    # copy queues in front of younger T-copies
                    if len(s_queue) > 1:
                        emit_scopy(s_queue.pop(0))
            while queue:
                qpg, qh, qts = queue.pop(0)
                emit_stage2(qpg, qh, qts)
                if qh == 1:
                    s_queue.append(qpg)
            while s_queue:
                emit_scopy(s_queue.pop(0))

    nc.compile()
    return nc


def _get_nc(b_local):
    if b_local not in _CACHE:
        _CACHE[b_local] = _build_nc(b_local)
    return _CACHE[b_local]


def _host_prep(W):
    """Derive the sign diagonal of the reference's QR and the masked W.

    Returns (wm, d) or (None, None) when W doesn't have orthonormal rows
    (then the closed form doesn't apply and the caller falls back)."""
    W = np.ascontiguousarray(W, dtype=np.float32)
    q, _ = np.linalg.qr(W.T)
    d = np.sign((q.T * W).sum(axis=1)).astype(np.float32)
    d[d == 0] = 1.0
    if np.abs(q.T - d[:, None] * W).max() >= 1e-4:
        return None, None
    wm = W * (d > 0).astype(np.float32)[:, None]
    return wm, d


def _reference_fallback(X, W):
    """Faithful numpy port of the reference (QR + eigh) — only used if the
    input W unexpectedly doesn't have orthonormal rows."""
    q, _ = np.linalg.qr(W.T.astype(np.float32))
    w_st = q.T
    y = np.einsum("mi,bij->bmj", w_st, X, optimize=True) @ W.T
    m = 0.5 * (y + y.transpose(0, 2, 1))
    lam, u = np.linalg.eigh(m)
    lam = np.maximum(lam, EPS)
    return np.einsum("bik,bk,bjk->bij", u, lam, u, optimize=True).astype(np.float32)


def run(X, W, trace=False, **trace_kwargs):
    X = np.ascontiguousarray(X, dtype=np.float32)
    wm, d = _host_prep(W)
    if wm is None:
        return _reference_fallback(X, W), None

    wh = wm.T.astype(np.float16)  # [128, 64] = W^T
    zz = np.zeros((H, D_OUT), dtype=np.float16)
    whe = np.concatenate([wh[H:D_IN], zz], axis=0)  # masks odd item
    who = np.concatenate([zz, wh[H:D_IN]], axis=0)  # masks even item
    wh3 = np.concatenate([wh, wh, whe, who], axis=1)  # [128, 256]

    # [B, i, b, j] i-major fp16
    xh = X.astype(np.float16)
    xh = xh.reshape(N_CORES, B_LOCAL, D_IN, D_IN).transpose(0, 2, 1, 3)
    # left block-column of L' (top 64x64 block halved; exact in fp16)
    xl = np.ascontiguousarray(xh[:, :, :, 0:H])
    xl[:, 0:H, :, :] *= np.float16(0.5)
    xl = xl.reshape(N_CORES, D_IN, B_LOCAL * H)
    # right-bottom block X22/2, item-pair packed onto 128 partitions
    xr = xh[:, H:D_IN, :, H:D_IN] * np.float16(0.5)  # [core, 64, b, 64]
    xr = xr.reshape(N_CORES, H, B_LOCAL // 2, 2, H).transpose(0, 3, 1, 2, 4)
    xr = np.ascontiguousarray(xr).reshape(N_CORES, D_IN, (B_LOCAL // 2) * H)

    from concourse.bass_utils import run_bass_kernel_spmd

    nc = _get_nc(B_LOCAL)
    in_maps = [
        {"XL": xl[c], "XR": xr[c], "WH3": wh3} for c in range(N_CORES)
    ]
    last_err = None
    for _attempt in range(3):
        try:
            res = run_bass_kernel_spmd(
                nc, in_maps, list(range(N_CORES)), trace=trace, **trace_kwargs
            )
            break
        except Exception as e:  # noqa: BLE001 - transient NRT device errors
            last_err = e
            import time

            time.sleep(2.0)
    else:
        raise last_err

    n_pair = B_LOCAL // PAIR
    z = np.empty((B_TOTAL, D_OUT, D_OUT), dtype=np.float32)
    for c in range(N_CORES):
        o = res.results[c]["OUT"].reshape(2, D_OUT, n_pair, GROUP, D_OUT)
        # o[h, m, pg, j, n] = Z[pg*16 + h*8 + j][m, n]
        o = o.transpose(2, 0, 3, 1, 4).reshape(B_LOCAL, D_OUT, D_OUT)
        z[c * B_LOCAL : (c + 1) * B_LOCAL] = o
    out = z + z.transpose(0, 2, 1)  # S = Z + Z^T (L' + L'^T = X)
    neg = d < 0
    if neg.any():
        idx = np.where(neg)[0]
        out[:, idx, idx] += EPS
    return out, res


def kernel(X, W):
    return run(X, W)[0]


# revision 32
# speedup vs baseline: 1.1641x; 1.0011x over previous
"""
nn_BiReBlock kernel for 8x Trainium2 NeuronCores.

Mathematical reduction (same as the verified baseline)
------------------------------------------------------
reference(X, W) with W having orthonormal rows reduces to
    out = Wm @ X @ Wm^T + eps * diag(1_N)
where Wm = W with QR-sign-negative rows zeroed (for the actual seed-0 W,
QR reproduces W exactly so Wm = W, N = {}).

Device computation (v8, "packed half-staircase")
------------------------------------------------
* fp16 is plenty (2e-2 budget vs ~4e-4 measured end-to-end), no residual.
* X is symmetric, so only the lower 2x2 block staircase of it is shipped:
  L' = [[X11/2, 0], [X21, X22/2]], which satisfies L' + L'^T = X.
  Since S = Wm X Wm^T is symmetric the device computes
      Z_b = Wm L'_b^T Wm^T
  and the host reconstructs S = Z + Z^T for free (75% of X shipped).
* The left block-column [128, 64] is a dense per-item stationary.
  The right-bottom blocks [64, 64] of consecutive item pairs are packed
  onto all 128 SBUF partitions (even item on 0:64, odd on 64:128), so
  every DMA runs with full 128-partition parallelism; the other item is
  masked out by a zero half in the W^T moving operand (sections 2/3 of
  WH3), so no SBUF zero-fills or row-tiled matmuls are needed and the
  PE's weight-load pull-ahead keeps streaming at full rate.
* stage 2 packs two 8-item groups into PSUM partitions 0:64 / 64:128
  via column tiling; it is emitted two half-groups late (software
  pipelining) so its wait for the PSUM->SBUF copy never stalls the
  in-order PE.  Copies convert to fp16 and alternate between the
  Vector and Scalar engines; output is fp16.
* A burst of dummy matmuls right after the first DMAs keeps the PE HAM
  activity window busy, so real work starts at the warm 2.4 GHz clock.

HBM traffic/core: 12 MB X + 4 MB out (vs 32 MB baseline).
"""

import numpy as np

B_TOTAL = 4096
N_CORES = 8
B_LOCAL = B_TOTAL // N_CORES
D_IN = 128
D_OUT = 64
EPS = 1e-4

_CACHE = {}

CHUNKS = [16, 16, 32, 64] + [96] * 4
assert sum(CHUNKS) == B_LOCAL
GROUP = 8
PAIR = 2 * GROUP          # items per packed stage-2 PSUM bank
OCH = 64                  # items per output flush
H = D_IN // 2             # 64
DEFER = 3                 # half-groups of stage-2 deferral


def _build_nc(b_local):
    import concourse.tile as tile
    from concourse import bacc, mybir

    f32 = mybir.dt.float32
    f16 = mybir.dt.float16
    nc = bacc.Bacc(None, target_bir_lowering=False)

    # left block-column of L', i-major: XL[i, b*64 + j] = L'_b[i, j]
    xld = nc.dram_tensor("XL", [D_IN, b_local * H], f16, kind="ExternalInput")
    # right-bottom blocks, item-pair packed:
    # XR[s*64 + p, q*64 + j] = (X22/2)_{2q+s}[p, j]
    xrd = nc.dram_tensor("XR", [D_IN, (b_local // 2) * H], f16,
                         kind="ExternalInput")
    # [W^T | W^T | [W^T[64:]; 0] | [0; W^T[64:]]] — the zero halves of the
    # last two sections mask out the other item of each packed XR pair
    wd = nc.dram_tensor("WH3", [D_IN, 4 * D_OUT], f16, kind="ExternalInput")
    n_pair = b_local // PAIR
    outd = nc.dram_tensor("OUT", [D_IN, n_pair * GROUP * D_OUT], f16,
                          kind="ExternalOutput")

    gfree = GROUP * D_OUT   # 512

    with tile.TileContext(nc) as tc:
        with (
            tc.tile_pool(name="const", bufs=1) as cpool,
            tc.tile_pool(name="tsb", bufs=7) as tpool,
            tc.tile_pool(name="obuf", bufs=3) as opool,
            tc.tile_pool(name="psum_t", bufs=4, space="PSUM") as pt,
            tc.tile_pool(name="psum_s", bufs=4, space="PSUM") as ps,
        ):
            wh3 = cpool.tile([D_IN, 4 * D_OUT], f16)
            nc.sync.dma_start(wh3[:], wd[:])

            # the whole per-core input fits in SBUF (96 KB/partition), so
            # keep it all resident and pre-issue every input DMA up front
            # in consumption order — no rotation, no WAR-gated prefetch
            xl = cpool.tile([D_IN, b_local, H], f16, name="xl")
            xr = cpool.tile([D_IN, b_local // 2, H], f16, name="xr")

            chunk_base = np.cumsum([0] + CHUNKS).tolist()
            for k in range(len(CHUNKS)):
                b0, n = chunk_base[k], CHUNKS[k]
                nc.sync.dma_start(
                    xl[:, b0 : b0 + n, :], xld[:, b0 * H : (b0 + n) * H]
                )
                nc.sync.dma_start(
                    xr[:, b0 // 2 : (b0 + n) // 2, :],
                    xrd[:, (b0 // 2) * H : ((b0 + n) // 2) * H],
                )

            # HAM warm-up: keep the PE busy on dummy matmuls while the
            # first chunks are still in flight (only wh3 is needed).
            warm = ps.tile([D_IN, gfree], f32, tag="sp", name="warm")
            for r in range(112):
                half = (r // 8) % 2
                nc.tensor.matmul(
                    warm[half * D_OUT : (half + 1) * D_OUT,
                         (r % 8) * D_OUT : (r % 8 + 1) * D_OUT],
                    wh3[:, 0:D_OUT],
                    wh3[:, 0:D_OUT],
                    start=True,
                    stop=True,
                    tile_position=(0, half * D_OUT),
                )

            obufs = {}
            sps = {}

            def emit_stage2(pg, h, ts):
                if h == 0:
                    sps[pg] = ps.tile([D_IN, gfree], f32, tag="sp", name="sp")
                sp = sps[pg]
                nc.tensor.matmul(
                    sp[h * D_OUT : (h + 1) * D_OUT, :],
                    wh3[:, h * D_OUT : (h + 1) * D_OUT],
                    ts[:],
                    start=True,
                    stop=True,
                    tile_position=(0, h * D_OUT),
                )

            def emit_scopy(pg):
                sp = sps.pop(pg)
                obuf = obufs[pg // (OCH // PAIR)]
                off = (pg % (OCH // PAIR)) * gfree
                scpy = nc.scalar.copy if pg % 2 == 0 else nc.vector.tensor_copy
                scpy(obuf[:, off : off + gfree], sp[:])
                c0 = pg * PAIR
                if (c0 + PAIR) % OCH == 0:
                    o0 = (pg // (OCH // PAIR)) * (OCH // PAIR) * gfree
                    olen = (OCH // PAIR) * gfree
                    if c0 + PAIR == b_local:
                        nc.gpsimd.dma_start(
                            outd[:, o0 : o0 + olen // 2], obuf[:, : olen // 2]
                        )
                        nc.gpsimd.dma_start(
                            outd[:, o0 + olen // 2 : o0 + olen],
                            obuf[:, olen // 2 :],
                        )
                    else:
                        nc.gpsimd.dma_start(outd[:, o0 : o0 + olen], obuf[:])

            queue = []    # (pg, h, ts) awaiting stage-2
            s_queue = []  # pg awaiting the sp -> obuf copy
            for pg in range(b_local // PAIR):
                c0 = pg * PAIR
                if c0 % OCH == 0:
                    obufs[pg // (OCH // PAIR)] = opool.tile(
                        [D_IN, (OCH // PAIR) * gfree], f16, tag="obuf",
                        name="obuf",
                    )
                for h in range(2):
                    tp = pt.tile([D_IN, gfree], f32, name="tp")
                    for j in range(GROUP):
                        b = c0 + h * GROUP + j
                        dst = tp[:, j * D_OUT : (j + 1) * D_OUT]
                        nc.tensor.matmul(
                            dst[0:H, :],
                            xl[:, b, :],
                            wh3[:, 0:D_OUT],
                            start=True,
                            stop=True,
                            tile_position=(0, 0),
                        )
                        s = b % 2
                        nc.tensor.matmul(
                            dst[H:D_IN, :],
                            xr[:, b // 2, :],
                            wh3[:, (2 + s) * D_OUT : (3 + s) * D_OUT],
                            start=True,
                            stop=True,
                            tile_position=(0, H),
                        )
                    ts = tpool.tile([D_IN, gfree], f16, tag="ts", name="ts")
                    cpy = (nc.vector.tensor_copy if (2 * pg + h) % 2 == 0
                           else nc.scalar.copy)
                    cpy(ts[:], tp[:])
                    queue.append((pg, h, ts))
                    if len(queue) > DEFER:
                        qpg, qh, qts = queue.pop(0)
                        emit_stage2(qpg, qh, qts)
                        if qh == 1:
                            s_queue.append(qpg)
                    # S-copies trail stage-2 by two more halves so a
                    # not-yet-ready S-copy never head-of-line-blocks the
                    # copy queues in front of younger T-copies
                    if len(s_queue) > 1:
                        emit_scopy(s_queue.pop(0))
            while queue:
                qpg, qh, qts = queue.pop(0)
                emit_stage2(qpg, qh, qts)
                if qh == 1:
                    s_queue.append(qpg)
            while s_queue:
                emit_scopy(s_queue.pop(0))

    nc.compile()
    return nc


def _get_nc(b_local):
    if b_local not in _CACHE:
        _CACHE[b_local] = _build_nc(b_local)
    return _CACHE[b_local]


def _host_prep(W):
    """Derive the sign diagonal of the reference's QR and the masked W.

    Returns (wm, d) or (None, None) when W doesn't have orthonormal rows
    (then the closed form doesn't apply and the caller falls back)."""
    W = np.ascontiguousarray(W, dtype=np.float32)
    q, _ = np.linalg.qr(W.T)
    d = np.sign((q.T * W).sum(axis=1)).astype(np.float32)
    d[d == 0] = 1.0
    if np.abs(q.T - d[:, None] * W).max() >= 1e-4:
        return None, None
    wm = W * (d > 0).astype(np.float32)[:, None]
    return wm, d


def _reference_fallback(X, W):
    """Faithful numpy port of the reference (QR + eigh) — only used if the
    input W unexpectedly doesn't have orthonormal rows."""
    q, _ = np.linalg.qr(W.T.astype(np.float32))
    w_st = q.T
    y = np.einsum("mi,bij->bmj", w_st, X, optimize=True) @ W.T
    m = 0.5 * (y + y.transpose(0, 2, 1))
    lam, u = np.linalg.eigh(m)
    lam = np.maximum(lam, EPS)
    return np.einsum("bik,bk,bjk->bij", u, lam, u, optimize=True).astype(np.float32)


def run(X, W, trace=False, **trace_kwargs):
    X = np.ascontiguousarray(X, dtype=np.float32)
    wm, d = _host_prep(W)
    if wm is None:
        return _reference_fallback(X, W), None

    wh = wm.T.astype(np.float16)  # [128, 64] = W^T
    zz = np.zeros((H, D_OUT), dtype=np.float16)
    whe = np.concatenate([wh[H:D_IN], zz], axis=0)  # masks odd item
    who = np.concatenate([zz, wh[H:D_IN]], axis=0)  # masks even item
    wh3 = np.concatenate([wh, wh, whe, who], axis=1)  # [128, 256]

    # [B, i, b, j] i-major fp16
    xh = X.astype(np.float16)
    xh = xh.reshape(N_CORES, B_LOCAL, D_IN, D_IN).transpose(0, 2, 1, 3)
    # left block-column of L' (top 64x64 block halved; exact in fp16)
    xl = np.ascontiguousarray(xh[:, :, :, 0:H])
    xl[:, 0:H, :, :] *= np.float16(0.5)
    xl = xl.reshape(N_CORES, D_IN, B_LOCAL * H)
    # right-bottom block X22/2, item-pair packed onto 128 partitions
    xr = xh[:, H:D_IN, :, H:D_IN] * np.float16(0.5)  # [core, 64, b, 64]
    xr = xr.reshape(N_CORES, H, B_LOCAL // 2, 2, H).transpose(0, 3, 1, 2, 4)
    xr = np.ascontiguousarray(xr).reshape(N_CORES, D_IN, (B_LOCAL // 2) * H)

    from concourse.bass_utils import run_bass_kernel_spmd

    nc = _get_nc(B_LOCAL)
    in_maps = [
        {"XL": xl[c], "XR": xr[c], "WH3": wh3} for c in range(N_CORES)
    ]
    last_err = None
    for _attempt in range(3):
        try:
            res = run_bass_kernel_spmd(
                nc, in_maps, list(range(N_CORES)), trace=trace, **trace_kwargs
            )
            break
        except Exception as e:  # noqa: BLE001 - transient NRT device errors
            last_err = e
            import time

            time.sleep(2.0)
    else:
        raise last_err

    n_pair = B_LOCAL // PAIR
    z = np.empty((B_TOTAL, D_OUT, D_OUT), dtype=np.float32)
    for c in range(N_CORES):
        o = res.results[c]["OUT"].reshape(2, D_OUT, n_pair, GROUP, D_OUT)
        # o[h, m, pg, j, n] = Z[pg*16 + h*8 + j][m, n]
        o = o.transpose(2, 0, 3, 1, 4).reshape(B_LOCAL, D_OUT, D_OUT)
        z[c * B_LOCAL : (c + 1) * B_LOCAL] = o
    out = z + z.transpose(0, 2, 1)  # S = Z + Z^T (L' + L'^T = X)
    neg = d < 0
    if neg.any():
        idx = np.where(neg)[0]
        out[:, idx, idx] += EPS
    return out, res


def kernel(X, W):
    return run(X, W)[0]
